# revision 41
# baseline (speedup 1.0000x reference)
"""Causal multi-head attention (B=2, S=2048, H=1024, 16 heads, hd=64) on 8
Trainium2 NeuronCores.

Sharding: batch x head-group. Core c handles batch c//4 and the 4 heads
4*(c%4)..4*(c%4)+3 (a 256-wide column slice of Q/K/V). Each core computes its
heads' contribution to the output projection (row-parallel Wo); the host sums
the 4 partials per batch and adds bo.

All matmul operands are bf16 (f32 PSUM accumulation): 1 cycle/row at any
moving width on the PE, so the causal staircase can be trimmed tile-by-tile.
The k-bias is dropped entirely: softmax over k is invariant to terms constant
in k, and (q+bq)@(k+bk)^T differs from (q+bq)@k^T by exactly such terms.

Per-core kernel:
  phase 1: qT/kT = W-slice^T.T @ xT (+bq via K=1 matmul; no bk), v natural
           (lhsT = xT chunk). xT = hidden[b].T prepared host-side; no
           on-device transposes. PSUM drained by ACT (q/k) and DVE (v).
  phase 2: per (head, 512-query block): scoresT[k,q] tiles on PE. Diagonal
           128-k-tiles j=0..3 are trimmed to columns [128j, 512) and the
           128-wide staircase wedge gets the mask added by an identity
           matmul (I.T @ mb accumulates mb into the open PSUM group), so
           the DVE never touches scores. exp on ACT (scale=1/8 folded; no
           max-subtraction needed), ctxT[65,q] = vaug.T @ expT with a ones
           column producing the softmax denominator in row 64. Softmax
           reciprocal on DVE (approx, ~51 ULP), broadcast across 64
           partitions by a K=1 matmul, applied by DVE into the outproj
           stationary layout.
           A global 2-unit software pipeline (scores of unit N+2 emitted
           before ctx of unit N) runs across slot boundaries, with norm
           and outproj units injected as PE filler after ctx pops: the PE
           instruction stream has no idle gaps, so the HAM clock gate
           stays at 8/8 (2.4 GHz) for the whole kernel.
  phase 3: out_partial[tok,1024] = ctxT.T @ WoT-slice per 128-token chunk,
           interleaved into the next query-block round as filler.
"""
from collections import deque

import numpy as np

import concourse.bass as bass
import concourse.mybir as mybir
import concourse.tile as tile
from concourse.bass import ts
from concourse.bass_utils import run_bass_kernel_spmd

B, S, H, NH, HD = 2, 2048, 1024, 16, 64
NCORES = 8
HPC = 4            # heads per core
HSW = HPC * HD     # 256: head-slice width
F32 = mybir.dt.float32
F32R = mybir.dt.float32r
BF16 = mybir.dt.bfloat16
NEG = -1.0e9
NQB = S // 512     # 4 query blocks per head
NTC = S // 128     # 16 token chunks


def _split_multi_waits(nc) -> int:
    """This walrus accepts at most ONE sync wait per instruction. Split any
    multi-wait instruction into single-wait NOPs (same engine, just before
    it) + the instruction carrying the last wait. Equivalent semantics:
    waits run in program order on the engine's queue."""
    n = 0
    for f in nc.m.functions:
        for blk in f.blocks:
            new_insts = []
            for inst in blk.instructions:
                si = inst.sync_info
                if si is not None and si.on_wait and len(si.on_wait) > 1:
                    waits = list(si.on_wait)
                    for i, w in enumerate(waits[:-1]):
                        new_insts.append(mybir.InstNoOp(
                            name=f"{inst.name}-ws{i}",
                            engine=inst.engine,
                            bass_nofuse=True,
                            sync_info=mybir.SyncInfo(on_wait=[w], on_update=[]),
                        ))
                        n += 1
                    si.on_wait = [waits[-1]]
                new_insts.append(inst)
            blk.instructions[:] = new_insts
    return n


def _build(debug=False):
    nc = bass.Bass()
    xt_d = nc.dram_tensor("xt", [H, S], BF16, kind="ExternalInput")
    wq_d = nc.dram_tensor("wq", [H, HSW], BF16, kind="ExternalInput")
    wk_d = nc.dram_tensor("wk", [H, HSW], BF16, kind="ExternalInput")
    wv_d = nc.dram_tensor("wv", [H, HSW], BF16, kind="ExternalInput")
    wo_d = nc.dram_tensor("wo", [HSW, H], BF16, kind="ExternalInput")
    bqv_d = nc.dram_tensor("bqv", [2, HSW], BF16, kind="ExternalInput")
    mb_d = nc.dram_tensor("mb", [128, 128], BF16, kind="ExternalInput")
    id_d = nc.dram_tensor("ident", [128, 128], BF16, kind="ExternalInput")
    out_d = nc.dram_tensor("out", [S, H], F32, kind="ExternalOutput")
    if debug:
        qT_d = nc.dram_tensor("qT_dbg", [128, 2, S], BF16, kind="ExternalOutput")
        kT_d = nc.dram_tensor("kT_dbg", [128, 2, S], BF16, kind="ExternalOutput")
        v_d = nc.dram_tensor("v_dbg", [128, 4, NTC, HD + 1], BF16,
                             kind="ExternalOutput")
        c_d = nc.dram_tensor("ctxT_dbg", [128, 2, S], BF16, kind="ExternalOutput")
        et_d = nc.dram_tensor("et_dbg", [128, 2, 512], BF16,
                              kind="ExternalOutput")

    EXP = mybir.ActivationFunctionType.Exp

    with tile.TileContext(nc) as tc:
        with tc.tile_pool(name="const", bufs=1) as constp, \
             tc.tile_pool(name="persist", bufs=1) as pers, \
             tc.tile_pool(name="xtp", bufs=1) as xtp, \
             tc.tile_pool(name="psum", bufs=2, space="PSUM") as psum, \
             tc.tile_pool(name="attnp", bufs=4) as attnp, \
             tc.tile_pool(name="recp", bufs=2) as recp, \
             tc.tile_pool(name="outp", bufs=3) as outp:
            wq = constp.tile([128, 8, HSW], BF16)
            wk = constp.tile([128, 8, HSW], BF16)
            wv = constp.tile([128, 8, HSW], BF16)
            wo = constp.tile([128, 2, H], BF16)
            bqv = constp.tile([1, 2, HSW], BF16)
            mb = constp.tile([128, 128], BF16)
            ident = constp.tile([128, 128], BF16)
            ones16 = constp.tile([1, 512], BF16)
            nc.vector.memset(ones16, 1.0)

            qT = pers.tile([128, 2, S], BF16)    # [2 heads x 64 hd, mc, tok]
            kT = pers.tile([128, 2, S], BF16)
            vaug = pers.tile([128, 4, NTC, HD + 1], BF16)  # [ktok, head, kchunk, hd|1]
            ctxT = pers.tile([128, 2, S], BF16)  # outproj stationary layout
            nc.vector.memset(vaug[:, :, :, HD:HD + 1], 1.0)

            # ---- DMA: weights on the ACT hwdge queue, xt on the SP queue,
            # both split fine-grained so the first matmul starts ~1.5us in.
            xt = xtp.tile([128, 8, S], BF16)
            for kc in range(8):
                nc.scalar.dma_start(out=wq[:, kc, :],
                                    in_=wq_d[ts(kc, 128), :])
            for kc in range(0, 8, 2):
                nc.sync.dma_start(out=xt[:, kc, ts(0, 512)],
                                  in_=xt_d[ts(kc, 128), ts(0, 512)])
            for kc in range(1, 8, 2):
                nc.scalar.dma_start(out=xt[:, kc, ts(0, 512)],
                                    in_=xt_d[ts(kc, 128), ts(0, 512)])
            nc.sync.dma_start(out=bqv, in_=bqv_d[:, :].rearrange("(o r) n -> o r n", o=1))
            nc.scalar.dma_start(out=wk, in_=wk_d[:, :].rearrange("(c p) n -> p c n", p=128))
            for nb in range(1, NQB):
                for kc in range(0, 8, 2):
                    nc.sync.dma_start(out=xt[:, kc, ts(nb, 512)],
                                      in_=xt_d[ts(kc, 128), ts(nb, 512)])
                for kc in range(1, 8, 2):
                    nc.scalar.dma_start(out=xt[:, kc, ts(nb, 512)],
                                        in_=xt_d[ts(kc, 128), ts(nb, 512)])
            nc.scalar.dma_start(out=wv, in_=wv_d[:, :].rearrange("(c p) n -> p c n", p=128))
            nc.scalar.dma_start(out=mb, in_=mb_d[:, :])
            nc.scalar.dma_start(out=ident, in_=id_d[:, :])
            nc.scalar.dma_start(out=wo, in_=wo_d[:, :].rearrange("(c p) n -> p c n", p=128))

            # ---- phase 1: all projections (q, k both head-pairs, v) ----
            # PSUM tags: "s" 2x[128,2,512] (4 banks), "c" 2x[128,512]
            # (2 banks), "o" 2x[128,512] (2 banks) = 8 banks total.
            for nb in range(NQB):
                for w, dst, mc, wb in ((wq, qT, 0, 1), (wq, qT, 1, 1),
                                       (wk, kT, 0, 0), (wk, kT, 1, 0)):
                    ps = psum.tile([128, 2, 512], F32, tag="s", name="ps")
                    for kc in range(8):
                        nc.tensor.matmul(ps[:, 0, :], w[:, kc, ts(mc, 128)],
                                         xt[:, kc, ts(nb, 512)],
                                         start=(kc == 0),
                                         stop=(kc == 7 and not wb))
                    if wb:
                        nc.tensor.matmul(ps[:, 0, :], bqv[0:1, 0, ts(mc, 128)],
                                         ones16[0:1, 0:512], start=False, stop=True)
                    nc.scalar.copy(out=dst[:, mc, ts(nb, 512)], in_=ps[:, 0, :])
                for t in range(4 * nb, 4 * nb + 4):
                    psv = psum.tile([128, 512], F32, tag="c", name="psv")
                    for kc in range(8):
                        nc.tensor.matmul(psv[:, 0:HSW], xt[:, kc, ts(t, 128)],
                                         wv[:, kc, :], start=(kc == 0), stop=False)
                    nc.tensor.matmul(psv[:, 0:HSW], ones16[0:1, 0:128],
                                     bqv[0:1, 1, :], start=False, stop=True)
                    nc.vector.tensor_copy(
                        out=vaug[:, :, t, 0:HD],
                        in_=psv[:, 0:HSW].rearrange("p (h d) -> p h d", h=HPC))

            # ---- phase 2+3: attention, globally software-pipelined ----
            fillers = deque()
            ctxq = deque()   # (emit_ctx_fn, on_done_fn | None)

            def make_norm_rec(qb, h, cps):
                # Softmax denominator sits in cps row 64: reciprocal on ACT
                # via exp(-ln(x)) - both functions live in one ACT table.
                rec = [None]

                def run():
                    lnr = recp.tile([1, 512], F32, tag="lnr", name="lnr")
                    nc.scalar.activation(out=lnr, in_=cps[HD:HD + 1, :],
                                         func=mybir.ActivationFunctionType.Ln)
                    rb = recp.tile([1, 512], BF16, tag="recb", name="rb")
                    nc.scalar.activation(out=rb, in_=lnr, func=EXP, scale=-1.0)
                    rec[0] = rb
                return rec, run

            def make_norm_apply(qb, h, cps, rec):
                def run():
                    mc, ro = h // 2, (h % 2) * HD
                    bps = psum.tile([64, 512], F32, tag="o", name="bps")
                    nc.tensor.matmul(bps, ones16[0:1, 0:64], rec[0][0:1, :],
                                     start=True, stop=True)
                    bsb = recp.tile([64, 512], BF16, tag="bsb", name="bsb")
                    nc.vector.tensor_copy(out=bsb, in_=bps)
                    nc.vector.tensor_mul(out=ctxT[ro:ro + HD, mc, ts(qb, 512)],
                                         in0=cps[0:HD, :], in1=bsb)
                return run

            def make_outproj(t, tail=False):
                def run():
                    osb = outp.tile([128, 2, 512], F32, tag="osb", name="osb")
                    for n2 in range(2):
                        ops = psum.tile([128, 512], F32, tag="o", name="ops")
                        nc.tensor.matmul(ops, ctxT[:, 0, ts(t, 128)],
                                         wo[:, 0, ts(n2, 512)],
                                         start=True, stop=False)
                        nc.tensor.matmul(ops, ctxT[:, 1, ts(t, 128)],
                                         wo[:, 1, ts(n2, 512)],
                                         start=False, stop=True)
                        # tail units split eviction across ACT+DVE so the
                        # final drain chain halves (ACT is idle by then).
                        if tail and n2 == 1:
                            nc.scalar.copy(out=osb[:, n2, :], in_=ops)
                        else:
                            nc.vector.tensor_copy(out=osb[:, n2, :], in_=ops)
                    nc.sync.dma_start(out=out_d[ts(t, 128), :], in_=osb)
                return run

            def pop_ctx():
                fn, on_done = ctxq.popleft()
                fn()
                # capture the filler BEFORE on_done so a freshly-queued
                # norm_apply runs one pop later: the PE broadcast matmul then
                # has a unit of slack behind the DVE reciprocal it waits on.
                fill = fillers.popleft() if fillers else None
                if on_done is not None:
                    on_done()
                if fill is not None:
                    fill()

            for qb in range(NQB):
                for h in range(HPC):
                    mc, ro = h // 2, (h % 2) * HD
                    cps = psum.tile([128, 512], F32, tag="c", name="cps")
                    nunits = 2 * qb + 2
                    rec, rec_run = make_norm_rec(qb, h, cps)
                    # Diagonal pairs FIRST (the zero-open makes accumulation
                    # order free): their small trimmed exps retire early and
                    # the slot's tail ctx waits only on big well-covered
                    # off-diagonal exps.
                    units = [2 * qb, 2 * qb + 1] + list(range(2 * qb))
                    for ui, g in enumerate(units):
                        diag = g >= 2 * qb
                        sps = psum.tile([128, 2, 512], F32, tag="s", name="sps")
                        et = attnp.tile([128, 2, 512], BF16, tag="et", name="et",
                                        bufs=5)
                        for u in range(2):
                            kb = 2 * g + u
                            j = kb - 4 * qb
                            lo = 128 * j if j >= 0 else 0
                            nc.tensor.matmul(
                                sps[:, u, lo:512],
                                kT[ro:ro + HD, mc, ts(kb, 128)],
                                qT[ro:ro + HD, mc, qb * 512 + lo:qb * 512 + 512],
                                start=True, stop=(j < 0))
                            if j >= 0:
                                # staircase wedge: accumulate mb via identity
                                nc.tensor.matmul(sps[:, u, lo:lo + 128],
                                                 ident, mb,
                                                 start=False, stop=True)
                        # One exp per unit. Diagonal pairs exp a merged range
                        # (stale sub-columns of the later tile are exp'd too
                        # but never read - sps banks only ever hold scores).
                        elo = 256 if (diag and g == 2 * qb + 1) else 0
                        nc.scalar.activation(out=et[:, :, elo:512],
                                             in_=sps[:, :, elo:512],
                                             func=EXP, scale=0.125)

                        if debug and qb == 0 and h == 0 and g == 0:
                            nc.sync.dma_start(out=et_d[:, :, :], in_=et)

                        def make_ctx(cps=cps, et=et, g=g, qb=qb, h=h,
                                     first=(ui == 0), final=(ui == nunits - 1)):
                            def run():
                                for u in range(2):
                                    kb = 2 * g + u
                                    j = kb - 4 * qb
                                    lo = 128 * j if j >= 0 else 0
                                    nc.tensor.matmul(cps[0:HD + 1, lo:512],
                                                     vaug[:, h, kb, :],
                                                     et[:, u, lo:512],
                                                     start=(first and u == 0),
                                                     stop=(final and u == 1))
                            return run

                        is_last = (ui == nunits - 1)
                        on_done = None
                        if is_last:
                            def on_done(qb=qb, h=h, cps=cps, rec=rec,
                                        rec_run=rec_run):
                                # ACT reciprocal right after the ctx stop; the
                                # PE broadcast + DVE scale go to the filler
                                # queue BACK so the PE never waits on them.
                                rec_run()
                                fillers.append(make_norm_apply(qb, h, cps, rec))
                                if h == 3:
                                    fillers.extend(
                                        make_outproj(t, tail=(qb == 3))
                                        for t in range(4 * qb, 4 * qb + 4))
                        ctxq.append((make_ctx(), on_done))
                        if len(ctxq) > 2:
                            pop_ctx()
            while ctxq:
                pop_ctx()
            while fillers:
                fillers.popleft()()
            if debug:
                nc.sync.dma_start(out=qT_d[:, :, :], in_=qT)
                nc.sync.dma_start(out=kT_d[:, :, :], in_=kT)
                nc.sync.dma_start(out=v_d[:, :, :, :], in_=vaug)
                nc.sync.dma_start(out=c_d[:, :, :], in_=ctxT)

    _split_multi_waits(nc)
    return nc


_NC_CACHE = []


def _get_nc():
    if not _NC_CACHE:
        _NC_CACHE.append(_build())
    return _NC_CACHE[0]


def _staircase_wedge() -> np.ndarray:
    """mb[p, w] = 0 where k<=q inside a diagonal 128-wide wedge, else NEG.
    For diagonal tile j (columns 128j..), allowed iff p <= w."""
    p = np.arange(128)[:, None]
    w = np.arange(128)[None, :]
    return np.where(p <= w, 0.0, NEG)


def _in_maps(inputs: dict) -> list[dict]:
    import ml_dtypes
    bf16 = ml_dtypes.bfloat16
    x = np.asarray(inputs["hidden_states"], dtype=np.float32)
    Wq = np.asarray(inputs["Wq"], dtype=np.float32)
    Wk = np.asarray(inputs["Wk"], dtype=np.float32)
    Wv = np.asarray(inputs["Wv"], dtype=np.float32)
    Wo = np.asarray(inputs["Wo"], dtype=np.float32)
    bq = np.asarray(inputs["bq"], dtype=np.float32)
    bv = np.asarray(inputs["bv"], dtype=np.float32)

    xts = [np.ascontiguousarray(x[b].T).astype(bf16) for b in range(B)]
    mb = _staircase_wedge().astype(bf16)
    ident = np.eye(128, dtype=np.float32).astype(bf16)
    maps = []
    for c in range(NCORES):
        b, hg = c // 4, c % 4
        hs = slice(hg * HSW, (hg + 1) * HSW)
        maps.append({
            "xt": xts[b],
            "wq": np.ascontiguousarray(Wq[hs, :].T).astype(bf16),
            "wk": np.ascontiguousarray(Wk[hs, :].T).astype(bf16),
            "wv": np.ascontiguousarray(Wv[hs, :].T).astype(bf16),
            "wo": np.ascontiguousarray(Wo[:, hs].T).astype(bf16),
            "bqv": np.ascontiguousarray(np.stack([bq[hs], bv[hs]])).astype(bf16),
            "mb": mb,
            "ident": ident,
        })
    return maps


def run(inputs: dict, **spmd_kwargs):
    """Returns (full_output, BassKernelResults)."""
    nc = _get_nc()
    res = run_bass_kernel_spmd(nc, _in_maps(inputs), list(range(NCORES)),
                               **spmd_kwargs)
    bo = np.asarray(inputs["bo"], dtype=np.float32)
    out = np.empty((B, S, H), dtype=np.float32)
    for b in range(B):
        acc = res.results[4 * b]["out"].astype(np.float32)
        for hg in range(1, 4):
            acc = acc + res.results[4 * b + hg]["out"]
        out[b] = acc + bo
    return out, res


def kernel(**inputs) -> np.ndarray:
    out, _ = run(inputs)
    return out
